# revision 60
# baseline (speedup 1.0000x reference)
"""Trainium2 Bass kernel for nn_DecoderModel_12352325943321.

6-layer post-LN decoder, SwiGLU FFN, per-head staggered windowed causal
attention (head h window (h+1)*64), tied-embedding lm_head.
B=2, S=1024, D=512, H=8, L=6, V=32000.

Sharding (8 NeuronCores): DP-2 over batch x sequence-sharded TP-4 within
each group (Korthikanti-style):
  - residual stream x lives sequence-sharded: rank r of a group owns tokens
    [r*256, (r+1)*256) of its batch, feature-major fp16 [512, 256].
  - per layer: AllGather x (1 collective) -> attention for the rank's 2
    heads (r, 7-r) over the full sequence -> partial Wo over the rank's
    head columns -> ReduceScatter back to sequence shards (1 collective)
    -> LN1, full-width SwiGLU FFN (replicated weights, local tokens), LN2
    all local.  2 collectives/layer instead of AG+AllReduce.
  - lm_head: vocab-sharded (rank owns rows [r*8000, (r+1)*8000)) after a
    final AllGather of the lnf output; host concat.

On-core layout: fp16 activations feature-major [D(part), T(free)], fp32
PSUM accumulation, weights host-pretransposed [K, M] (fp16 except the FFN
weights, which are fp8 e4m3 scaled 256x with the unscale folded into the
existing activation/bias ops, and consumed by DoubleRow fp8 matmuls over
k-tile pairs).  Attention masks are additive (0 / -60000) and preloaded
into PSUM by an identity matmul so the score matmul accumulates onto them;
the two head slots' (score, exp, PV) pipelines are interleaved with each
PV deferred two slots so the in-order PE queue never stalls on exp.
LayerNorm reduces mean/sumsq with a ones-column matmul, takes rstd via
exp(-0.5*ln(var+eps)) (same act table as attention's Exp -> no table
reloads), and broadcasts [rstd | -mu*rstd] with gamma folded in via a
1-contraction PE matmul; the apply and both residual adds use the fused
DVE affine_then_add.  All inputs are packed into three flat tensors (fp16/
fp32/fp8) because the exec stack pays a per-input-tensor cost every call.
Per-layer weight DMAs are prefetched at the top of the layer on the sync
queue; collective-adjacent DMAs ride the scalar queue so neither blocks
the other.  The SPMD graph is rank-independent: all per-rank differences
(head windows, weight slices, masks, token shards) live in input data, so
every core executes the same NEFF.
"""
import os
from contextlib import ExitStack
import numpy as np
import ml_dtypes

import concourse.bass as bass
import concourse.bacc as bacc
import concourse.tile as tile
import concourse.mybir as mybir
from concourse import bass_utils
from concourse.masks import make_identity

f32 = mybir.dt.float32
f16 = mybir.dt.float16
f8 = mybir.dt.float8e4
bf16 = mybir.dt.bfloat16
i32 = mybir.dt.int32
AF = mybir.ActivationFunctionType
ALU = mybir.AluOpType

B, S, D, H, L, V, W, FF = 2, 1024, 512, 8, 6, 32000, 64, 2048
HD = D // H          # 64
TP = 4               # tensor-parallel ranks per group
NC = 8
TS = S // TP         # 256 tokens owned per rank
VS = V // TP         # 8000 vocab rows per rank
NT = S // 128        # 8 token tiles (full seq)
ND = D // 128        # 4 feature tiles
NSP = S // 512       # 2 token spans of 512
NH = 2 * FF // 128   # 32 hidden tiles (u+g)
NK2 = FF // 128      # 16 k-tiles for W2
NVS = VS // 500      # 16 vocab subtiles
EPS = 1e-5

# fixed attention (span, ktile) schedule per head slot; A: win<=256, B: <=512
SLOT_KT = {
    0: [(0, t) for t in range(0, 4)] + [(1, t) for t in range(2, 8)],
    1: [(0, t) for t in range(0, 4)] + [(1, t) for t in range(0, 8)],
}
NMASK = len(SLOT_KT[0]) + len(SLOT_KT[1])  # 22

_CACHE = {}
LAST_RESULTS = None
GP = False  # offload mask/residual/hb/t1 elementwise to gpsimd (else vector)
NOCOLL = False  # timing experiment: replace collectives with local DMA copies


# Input packing: the execution stack pays a per-ExternalInput-tensor cost on
# EVERY exec (~35-40us each through the PJRT/axon path), so all inputs are
# packed into two flat tensors (one per dtype) and carved up with AP views.
def _pack_spec(n_layers):
    spec16 = [
        ("x0", (D, TS)), ("embT_lm", (D, VS)),
        ("woT", (n_layers, 128, D)), ("mask", (NMASK, 128, 512)),
        ("wqkvT", (n_layers, D, 384)), ("ln1g_row", (n_layers, 1, D)),
        ("ln2g_row", (n_layers, 1, D)), ("lnfg_row", (1, D)),
    ]
    spec32 = [
        ("bqk", (n_layers, 256, 1)), ("vbias", (n_layers, 128, 128)),
        ("bo", (n_layers, D, 1)), ("b1", (n_layers, 2 * FF, 1)),
        ("b2", (n_layers, D, 1)), ("ln1g", (n_layers, D, 1)),
        ("ln1b", (n_layers, D, 1)), ("ln2g", (n_layers, D, 1)),
        ("ln2b", (n_layers, D, 1)), ("lnfg", (D, 1)), ("lnfb", (D, 1)),
    ]
    # fp8 FFN weights, host-scaled by 256 (e4m3, unscale folded downstream)
    spec8 = [
        ("w1T", (n_layers, D, 2 * FF)), ("w2T", (n_layers, FF, D)),
    ]
    return spec16, spec32, spec8


def _carve(base_ap, spec):
    aps = {}
    off = 0
    for nm, shp in spec:
        n = int(np.prod(shp))
        if n == 0:
            aps[nm] = None
            continue
        ap = base_ap[off:off + n]
        if len(shp) > 1:
            letters = list("abcde"[:len(shp)])
            pat = f"({' '.join(letters)}) -> {' '.join(letters)}"
            ap = ap.rearrange(pat, **{letters[i]: int(shp[i])
                                      for i in range(len(shp) - 1)})
        aps[nm] = ap
        off += n
    return aps


# ----------------------------------------------------------------- build
def build_nc(n_layers=L, taps=()):
    nc = bacc.Bacc("TRN2", target_bir_lowering=False, debug=False,
                   enable_asserts=True, num_devices=NC)

    spec16, spec32, spec8 = _pack_spec(n_layers)
    tot16 = sum(int(np.prod(shp)) for _, shp in spec16)
    tot32 = sum(int(np.prod(shp)) for _, shp in spec32)
    tot8 = sum(int(np.prod(shp)) for _, shp in spec8)
    aps16 = _carve(nc.dram_tensor("pk16", [tot16], f16,
                                  kind="ExternalInput").ap(), spec16)
    aps32 = _carve(nc.dram_tensor("pk32", [tot32], f32,
                                  kind="ExternalInput").ap(), spec32)
    aps8 = _carve(nc.dram_tensor("pk8", [max(tot8, 1)], f8,
                                 kind="ExternalInput").ap(), spec8)

    E = {
        "n_layers": n_layers,
        "groups": [[0, 1, 2, 3], [4, 5, 6, 7]],
        "x0_ap": aps16["x0"],
        "embT_ap": aps16["embT_lm"],
        "wqkv_ap": aps16["wqkvT"],
        "bqk_ap": aps32["bqk"],
        "vbias_ap": aps32["vbias"],
        "wo_ap": aps16["woT"],
        "bo_ap": aps32["bo"],
        "w1_ap": aps8["w1T"],
        "b1_ap": aps32["b1"],
        "w2_ap": aps8["w2T"],
        "b2_ap": aps32["b2"],
        "ln1g_ap": aps32["ln1g"],
        "ln1b_ap": aps32["ln1b"],
        "ln2g_ap": aps32["ln2g"],
        "ln2b_ap": aps32["ln2b"],
        "lnfg_ap": aps32["lnfg"],
        "lnfb_ap": aps32["lnfb"],
        "mask_ap": aps16["mask"],
        "ln1gr_ap": aps16["ln1g_row"],
        "ln2gr_ap": aps16["ln2g_row"],
        "lnfgr_ap": aps16["lnfg_row"],
        "out_ap": nc.dram_tensor("logits", [S, VS], f16, kind="ExternalOutput").ap(),
        "tap_aps": {t: nc.dram_tensor(f"tap_{t}", [D, TS], f32,
                                      kind="ExternalOutput").ap() for t in taps},
        "ag_in": [nc.dram_tensor(f"ag_in{l}", [D, TS],
                                 f8 if l < n_layers else f16,
                                 kind="Internal").ap()
                  for l in range(n_layers + 1)],
        "ag_out": [nc.dram_tensor(f"ag_out{l}", [TP * D, TS],
                                  f8 if l < n_layers else f16,
                                  kind="Internal").ap()
                   for l in range(n_layers + 1)],
        "rs_in": [nc.dram_tensor(f"rs_in{l}", [TP * D, TS], f16,
                                 kind="Internal").ap()
                  for l in range(n_layers)],
        "rs_out": [nc.dram_tensor(f"rs_out{l}", [D, TS], f16,
                                  kind="Internal").ap()
                   for l in range(n_layers)],
    }

    with tile.TileContext(nc) as tc:
        _emit(tc, E)
    nc.compile()
    return nc


def _emit(tc, E):
    with ExitStack() as _ctx:
        _emit_body(tc, E, _ctx)


def _emit_body(tc, E, ctx):
    nc = tc.nc
    n_layers = E["n_layers"]
    taps = E["tap_aps"]

    const = ctx.enter_context(tc.tile_pool(name="const", bufs=1))
    resid = ctx.enter_context(tc.tile_pool(name="resid", bufs=1))
    wpool = ctx.enter_context(tc.tile_pool(name="wpool", bufs=2))
    act = ctx.enter_context(tc.tile_pool(name="act", bufs=2))   # small transients
    big = ctx.enter_context(tc.tile_pool(name="big", bufs=1))   # per-layer tensors
    lnp = ctx.enter_context(tc.tile_pool(name="lnp", bufs=1))
    ps = ctx.enter_context(tc.tile_pool(name="ps", bufs=2, space="PSUM"))
    ps_att = ctx.enter_context(tc.tile_pool(name="ps_att", bufs=3, space="PSUM"))
    ps_pv = ctx.enter_context(tc.tile_pool(name="ps_pv", bufs=1, space="PSUM"))
    ps_sm = ctx.enter_context(tc.tile_pool(name="ps_sm", bufs=1, space="PSUM"))

    # ---------------- constants
    ident16 = const.tile([128, 128], f16, name="ident16")
    make_identity(nc, ident16[:])
    ones_col = const.tile([128, 1], f32, name="ones_col")
    nc.any.memset(ones_col[:], 1.0)
    ones_row = const.tile([1, 128], f16, name="ones_row")
    nc.any.memset(ones_row[:], 1.0)
    eps1 = const.tile([1, 1], f32, name="eps1")
    nc.any.memset(eps1[:], EPS)
    masks = [const.tile([128, 512], f16, name=f"mask{i}") for i in range(NMASK)]
    for i in range(NMASK):
        nc.scalar.dma_start(masks[i][:], E["mask_ap"][i])

    def load_dvec(ap_2d, pool, name, width=ND):
        t = pool.tile([128, width], f32, name=name, tag=name)
        nc.sync.dma_start(t[:], ap_2d.rearrange("(d p) one -> p (d one)", p=128))
        return t

    lnf_gr = const.tile([1, D], f16, name="lnf_gr")
    nc.sync.dma_start(lnf_gr[:], E["lnfgr_ap"][:])
    lnf_b = load_dvec(E["lnfb_ap"], const, "lnf_b")

    # residual stream: fp16 feature-major, local 256 tokens, in ONE tile so
    # adjacent d-tiles can be pair-sliced for DoubleRow fp8 matmuls
    # (embedding gather + posenc precomputed on host as x0 [D, TS] fp16)
    x_all = resid.tile([128, ND * TS], f16, name="x_all")
    x = [x_all[:, d * TS:(d + 1) * TS] for d in range(ND)]
    for d in range(ND):
        nc.sync.dma_start(x[d], E["x0_ap"][d * 128:(d + 1) * 128, :])

    # ---------------- local feature-major LayerNorm (updates x in place)
    # grow: gamma as an f16 row [1, D]; bvec: beta [128, ND] f32
    def ln_inplace(grow, bvec):
        fold2 = lnp.tile([128, 2 * TS], f32, name="fold2", tag="fold2")
        fold = fold2[:, 0:TS]
        sqf = fold2[:, TS:2 * TS]
        f01 = lnp.tile([128, TS], f32, name="f01", tag="f01")
        nc.vector.tensor_tensor(out=f01[:], in0=x[0][:], in1=x[1][:], op=ALU.add)
        nc.vector.tensor_tensor(out=fold, in0=x[2][:], in1=x[3][:], op=ALU.add)
        nc.vector.tensor_tensor(out=fold, in0=fold, in1=f01[:], op=ALU.add)
        # squares on the Act engine (Square shares the exp/ln table -> no
        # table reload); sums of squares folded on DVE
        nc.scalar.activation(sqf, x[0][:], AF.Square)
        sq = lnp.tile([128, TS], f32, name="sq", tag="sq")
        sq2 = lnp.tile([128, TS], f32, name="sq2", tag="sq2")
        nc.scalar.activation(sq[:], x[1][:], AF.Square)
        nc.vector.tensor_tensor(out=sqf, in0=sqf, in1=sq[:], op=ALU.add)
        nc.scalar.activation(sq2[:], x[2][:], AF.Square)
        nc.scalar.activation(sq[:], x[3][:], AF.Square)
        nc.vector.tensor_tensor(out=sq2[:], in0=sq2[:], in1=sq[:], op=ALU.add)
        nc.vector.tensor_tensor(out=sqf, in0=sqf, in1=sq2[:], op=ALU.add)
        st = ps_sm.tile([1, 2 * TS], f32, name="st", tag="sm")
        nc.tensor.matmul(out=st[:], lhsT=ones_col[:], rhs=fold2[:],
                         start=True, stop=True)
        # negated mean so the apply tail is x*(rstd*g) + (-m*rstd*g) + b
        mn = lnp.tile([1, 2 * TS], f32, name="mn", tag="mn")
        nc.vector.tensor_scalar(out=mn[:, 0:TS], in0=st[:, 0:TS],
                                scalar1=-1.0 / D, scalar2=None, op0=ALU.mult)
        nc.vector.tensor_scalar(out=mn[:, TS:2 * TS], in0=st[:, TS:2 * TS],
                                scalar1=1.0 / D, scalar2=None, op0=ALU.mult)
        mean_neg = mn[:, 0:TS]
        var = mn[:, TS:2 * TS]
        msq = lnp.tile([1, TS], f32, name="msq", tag="msq")
        nc.vector.tensor_tensor(out=msq[:], in0=mean_neg, in1=mean_neg,
                                op=ALU.mult)
        nc.vector.tensor_tensor(out=var, in0=var, in1=msq[:], op=ALU.subtract)
        # rstd = (var+eps)^-0.5 via exp(-0.5*ln(var+eps)): Ln and Exp live in
        # the same activation table as attention's Exp -> no table reloads
        lnv = lnp.tile([1, TS], f32, name="lnv", tag="lnv")
        nc.scalar.activation(lnv[:], var, AF.Ln, bias=eps1[:], scale=1.0)
        rstd = lnp.tile([1, TS], f32, name="rstd", tag="rstd")
        nc.scalar.activation(rstd[:], lnv[:], AF.Exp, scale=-0.5)
        rm16 = lnp.tile([1, 2 * TS], f16, name="rm16", tag="rm16")
        nc.vector.tensor_copy(out=rm16[:, 0:TS], in_=rstd[:])
        nc.vector.tensor_tensor(out=rm16[:, TS:2 * TS], in0=mean_neg,
                                in1=rstd[:], op=ALU.mult)
        # per-d-tile broadcast with gamma folded in: a 1-contraction PE
        # matmul gives bcg = gamma_d (x) [rstd | -m*rstd]; the apply is then
        # two DVE ops: t1 = x*bcg_lo; x = (t1 + beta) + bcg_hi
        for d in range(ND):
            bcg = ps.tile([128, 2 * TS], f32, name="bcg", tag="mm")
            nc.tensor.matmul(out=bcg[:], lhsT=grow[:, d * 128:(d + 1) * 128],
                             rhs=rm16[:], start=True, stop=True)
            t1 = lnp.tile([128, TS], f16, name="t1", tag="t1")
            nc.vector.tensor_tensor(out=t1[:], in0=x[d][:], in1=bcg[:, 0:TS],
                                    op=ALU.mult)
            nc.vector.affine_then_add(out=x[d], in0=t1[:],
                                      in1=bcg[:, TS:2 * TS], scale=1.0,
                                      bias=bvec[:, d:d + 1])

    def tap(name):
        if name not in taps:
            return
        for d in range(ND):
            tf = lnp.tile([128, TS], f32, name="tapf", tag="t1")
            nc.vector.tensor_copy(out=tf[:], in_=x[d][:])
            nc.sync.dma_start(taps[name][d * 128:(d + 1) * 128, :], tf[:])

    def allgather_x(l, fp8):
        # all DMAs here ride the scalar queue: they gate (or are gated by) the
        # collective, and must not sit behind weight prefetches on sync
        if fp8:
            for d in range(ND):
                x8 = act.tile([128, TS], f8, name="x8", tag="x8")
                nc.vector.tensor_copy(out=x8[:], in_=x[d][:])
                nc.scalar.dma_start(E["ag_in"][l][d * 128:(d + 1) * 128, :], x8[:])
        else:
            for d in range(ND):
                nc.scalar.dma_start(E["ag_in"][l][d * 128:(d + 1) * 128, :], x[d][:])
        if NOCOLL:
            for rr in range(TP):
                nc.scalar.dma_start(E["ag_out"][l][rr * D:(rr + 1) * D, :],
                                    E["ag_in"][l][:])
        else:
            nc.gpsimd.collective_compute(
                "AllGather", ALU.bypass, replica_groups=E["groups"],
                ins=[E["ag_in"][l][:].opt()], outs=[E["ag_out"][l][:].opt()])
        xg = [big.tile([128, S], f16, name=f"xg{d}", tag=f"xg{d}")
              for d in range(ND)]
        src = E["ag_out"][l].rearrange("(r f) t -> f r t", r=TP)
        for d in range(ND):
            if fp8:
                xg8 = big.tile([128, S], f8, name=f"xg8_{d}", tag=f"xg8_{d}")
                nc.scalar.dma_start(
                    xg8[:].rearrange("p (r t) -> p r t", r=TP),
                    src[d * 128:(d + 1) * 128])
                nc.vector.tensor_copy(out=xg[d][:], in_=xg8[:])
            else:
                nc.scalar.dma_start(
                    xg[d][:].rearrange("p (r t) -> p r t", r=TP),
                    src[d * 128:(d + 1) * 128])
        return xg

    # ---------------- layers
    for l in range(n_layers):
        # prefetch ALL layer params first, on the sync queue (keeps the queue
        # free of collective-dependent loads so weight DMAs never stall)
        wqkv = wpool.tile([128, ND, 384], f16, name="wqkv", tag="wqkv")
        nc.sync.dma_start(wqkv[:], E["wqkv_ap"][l].rearrange("(k p) m -> p k m", p=128))
        bqk = wpool.tile([128, 2], f32, name="bqk", tag="bqk")
        nc.sync.dma_start(bqk[:], E["bqk_ap"][l].rearrange("(a p) one -> p (a one)", p=128))
        vbias = wpool.tile([128, 128], f32, name="vbias", tag="vbias")
        nc.sync.dma_start(vbias[:], E["vbias_ap"][l])
        wo = wpool.tile([128, D], f16, name="wo", tag="wo")
        nc.sync.dma_start(wo[:], E["wo_ap"][l])
        bo_t = load_dvec(E["bo_ap"][l], wpool, "bo_t")
        ln1gr = wpool.tile([1, D], f16, name="ln1gr", tag="ln1gr")
        nc.sync.dma_start(ln1gr[:], E["ln1gr_ap"][l])
        ln1b = load_dvec(E["ln1b_ap"][l], wpool, "ln1b")
        w1 = wpool.tile([128, ND, 2 * FF], f8, name="w1", tag="w1", bufs=1)
        nc.sync.dma_start(w1[:], E["w1_ap"][l].rearrange("(k p) m -> p k m", p=128))
        b1 = load_dvec(E["b1_ap"][l], wpool, "b1", width=NH)
        w2 = wpool.tile([128, NK2, D], f8, name="w2", tag="w2", bufs=1)
        nc.sync.dma_start(w2[:], E["w2_ap"][l].rearrange("(k p) m -> p k m", p=128))
        b2 = load_dvec(E["b2_ap"][l], wpool, "b2")
        ln2gr = wpool.tile([1, D], f16, name="ln2gr", tag="ln2gr")
        nc.sync.dma_start(ln2gr[:], E["ln2gr_ap"][l])
        ln2b = load_dvec(E["ln2b_ap"][l], wpool, "ln2b")

        xg = allgather_x(l, fp8=True)
        DR = mybir.MatmulPerfMode.DoubleRow

        # q, k feature-major [128, S] fp16 (rows: slotA 0:64, slotB 64:128)
        q_sb = big.tile([128, S], f16, name="q_sb", tag="q_sb")
        k_sb = big.tile([128, S], f16, name="k_sb", tag="k_sb")
        for mi, dest in ((0, q_sb), (1, k_sb)):
            for sp in range(NSP):
                sl = slice(sp * 512, (sp + 1) * 512)
                pm = ps.tile([128, 512], f32, name="pm_qk", tag="mm")
                for k in range(ND):
                    nc.tensor.matmul(out=pm[:],
                                     lhsT=wqkv[:, k, mi * 128:(mi + 1) * 128],
                                     rhs=xg[k][:, sl],
                                     start=(k == 0), stop=(k == ND - 1))
                nc.scalar.activation(dest[:, sl], pm[:], AF.Identity,
                                     bias=bqk[:, mi:mi + 1])

        # v token-major per tok-tile: [128, 130] = [vA(64) | 1 | vB(64) | 1]
        vts = []
        for t in range(NT):
            pv = ps.tile([128, 128], f32, name="pv_v", tag="mm")
            for k in range(ND):
                nc.tensor.matmul(out=pv[:], lhsT=xg[k][:, t * 128:(t + 1) * 128],
                                 rhs=wqkv[:, k, 256:384],
                                 start=(k == 0), stop=(k == ND - 1))
            vsb = big.tile([128, 130], f16, name=f"v65_{t}", tag=f"v65_{t}")
            nc.vector.tensor_tensor(out=vsb[:, 0:64], in0=pv[:, 0:64],
                                    in1=vbias[:, 0:64], op=ALU.add)
            nc.vector.tensor_tensor(out=vsb[:, 65:129], in0=pv[:, 64:128],
                                    in1=vbias[:, 64:128], op=ALU.add)
            nc.any.memset(vsb[:, 64:65], 1.0)
            nc.any.memset(vsb[:, 129:130], 1.0)
            vts.append(vsb)

        # attention: fixed 22 k-tile schedule; 0/1 masks multiply exp(scores).
        # The two head slots' accumulation groups are interleaved and each PV
        # matmul is deferred two schedule slots so the in-order PE queue does
        # other tiles' score matmuls while exp+mask of this tile complete.
        a_sb = big.tile([128, S], f16, name="a_sb", tag="a_sb")
        mask_id = {}
        mi_idx = 0
        for slot in (0, 1):
            for sp, t in SLOT_KT[slot]:
                mask_id[(slot, sp, t)] = mi_idx
                mi_idx += 1

        def attn_span(sp):
            # interleaved tile sequence for the two slots on this span
            seqs = []
            for slot in (0, 1):
                kts = [t for s_, t in SLOT_KT[slot] if s_ == sp]
                seqs.append([(slot, t, i, len(kts)) for i, t in enumerate(kts)])
            order = []
            i = j = 0
            while i < len(seqs[0]) or j < len(seqs[1]):
                if i < len(seqs[0]):
                    order.append(seqs[0][i]); i += 1
                if j < len(seqs[1]):
                    order.append(seqs[1][j]); j += 1
            qsl = slice(sp * 512, (sp + 1) * 512)
            pvp = {s: ps_pv.tile([65, 512], f32, name=f"pvp{s}", tag=f"pvp{s}")
                   for s in (0, 1)}
            pending = []

            def flush_one():
                slot, t, i, n, p_sb = pending.pop(0)
                nc.tensor.matmul(out=pvp[slot][:],
                                 lhsT=vts[t][:, slot * 65:slot * 65 + 65],
                                 rhs=p_sb[:], start=(i == 0), stop=(i == n - 1))

            for slot, t, i, n in order:
                rows = slice(slot * 64, slot * 64 + 64)
                scp = ps_att.tile([128, 512], f32, name="scp", tag="scp")
                # additive mask (0 / -60000) preloaded into PSUM by an
                # identity matmul; the score matmul accumulates onto it
                nc.tensor.matmul(out=scp[:], lhsT=ident16[:],
                                 rhs=masks[mask_id[(slot, sp, t)]][:],
                                 start=True, stop=False)
                nc.tensor.matmul(out=scp[:], lhsT=k_sb[rows, t * 128:(t + 1) * 128],
                                 rhs=q_sb[rows, qsl], start=False, stop=True)
                p_sb = act.tile([128, 512], f16, name="p_sb", tag="p_sb", bufs=4)
                nc.scalar.activation(p_sb[:], scp[:], AF.Exp)
                pending.append((slot, t, i, n, p_sb))
                if len(pending) > 2:
                    flush_one()
            while pending:
                flush_one()
            for slot in (0, 1):
                rows = slice(slot * 64, slot * 64 + 64)
                den = act.tile([1, 512], f32, name="den", tag="den")
                nc.vector.reciprocal(out=den[:], in_=pvp[slot][64:65, :])
                den_b = act.tile([64, 512], f32, name="den_b", tag="den_b")
                nc.gpsimd.partition_broadcast(den_b[:], den[:])
                nc.vector.tensor_tensor(out=a_sb[rows, qsl], in0=pvp[slot][0:64, :],
                                        in1=den_b[:], op=ALU.mult)

        for sp in range(NSP):
            attn_span(sp)

        # partial Wo over this rank's head columns -> ReduceScatter
        for m in range(ND):
            for sp in range(NSP):
                sl = slice(sp * 512, (sp + 1) * 512)
                pm = ps.tile([128, 512], f32, name="pm_wo", tag="mm")
                nc.tensor.matmul(out=pm[:], lhsT=wo[:, m * 128:(m + 1) * 128],
                                 rhs=a_sb[:, sl], start=True, stop=True)
                fsb = act.tile([128, 512], f16, name="fsb", tag="fsb")
                nc.vector.tensor_copy(out=fsb[:], in_=pm[:])
                dst = E["rs_in"][l].rearrange("(dest f) t -> f dest t", dest=TP)
                # sync queue: layer-l weight prefetches have drained by now,
                # and keeping these off the scalar queue frees the Act engine
                nc.sync.dma_start(
                    dst[m * 128:(m + 1) * 128, 2 * sp:2 * sp + 2],
                    fsb[:].rearrange("p (h t) -> p h t", h=2))
        if NOCOLL:
            nc.scalar.dma_start(E["rs_out"][l][:], E["rs_in"][l][0:D, :])
        else:
            nc.gpsimd.collective_compute(
                "ReduceScatter", ALU.add, replica_groups=E["groups"],
                ins=[E["rs_in"][l][:].opt()], outs=[E["rs_out"][l][:].opt()])

        rt4 = act.tile([128, ND * TS], f16, name="rt4", tag="rt4")
        nc.sync.dma_start(rt4[:].rearrange("p (d t) -> p d t", d=ND),
                          E["rs_out"][l].rearrange("(d f) t -> f d t", d=ND))
        for d in range(ND):
            # x += rt4_d + bo in one fused DVE op
            nc.vector.affine_then_add(out=x[d], in0=rt4[:, d * TS:(d + 1) * TS],
                                      in1=x[d], scale=1.0,
                                      bias=bo_t[:, d:d + 1])

        tap(f"res1_{l}")
        ln_inplace(ln1gr, ln1b)
        tap(f"ln1_{l}")

        # FFN: full hidden width on local tokens, hidden-major.  fp8x256
        # weights, fp8 activations (x re-quantized post-LN1), DoubleRow over
        # k-pairs.  Scale ledger: pu/pg = 256x; usb = 16x (scale 1/16, b1u
        # host-prescaled 16x); gsb = true (scale 1/256); hb = 16x fp8;
        # pf = 256*16x -> ot = pf/4096 + b2.
        x8f = big.tile([128, ND * TS], f8, name="x8f", tag="x8f")
        for d in range(ND):
            nc.vector.tensor_copy(out=x8f[:, d * TS:(d + 1) * TS], in_=x[d])
        x83 = x8f[:].rearrange("p (d t) -> p d t", d=ND)
        hb_all = big.tile([128, NK2 * TS], f8, name="hb_all")
        hb3 = hb_all[:].rearrange("p (k t) -> p k t", k=NK2)
        for m in range(NK2):
            pu = ps.tile([128, TS], f32, name="pu", tag="mm")
            for j in range(ND // 2):
                nc.tensor.matmul(out=pu[:],
                                 lhsT=w1[:, 2 * j:2 * j + 2, m * 128:(m + 1) * 128],
                                 rhs=x83[:, 2 * j:2 * j + 2, :],
                                 start=(j == 0), stop=(j == ND // 2 - 1),
                                 perf_mode=DR)
            pg = ps.tile([128, TS], f32, name="pg", tag="mm")
            for j in range(ND // 2):
                nc.tensor.matmul(out=pg[:],
                                 lhsT=w1[:, 2 * j:2 * j + 2,
                                          FF + m * 128:FF + (m + 1) * 128],
                                 rhs=x83[:, 2 * j:2 * j + 2, :],
                                 start=(j == 0), stop=(j == ND // 2 - 1),
                                 perf_mode=DR)
            usb = act.tile([128, TS], f16, name="usb", tag="usb")
            if m % 2 == 0:
                nc.scalar.activation(usb[:], pu[:], AF.Identity,
                                     bias=b1[:, m:m + 1], scale=1.0 / 16)
            else:
                nc.vector.tensor_scalar(out=usb[:], in0=pu[:], scalar1=1.0 / 16,
                                        scalar2=b1[:, m:m + 1], op0=ALU.mult,
                                        op1=ALU.add)
            gsb = act.tile([128, TS], f16, name="gsb", tag="gsb")
            nc.scalar.activation(gsb[:], pg[:], AF.Silu,
                                 bias=b1[:, NK2 + m:NK2 + m + 1], scale=1.0 / 256)
            (nc.gpsimd if GP else nc.vector).tensor_tensor(
                out=hb3[:, m, :], in0=usb[:], in1=gsb[:], op=ALU.mult)
        for m in range(ND):
            pf = ps.tile([128, TS], f32, name="pf", tag="mm")
            for j in range(NK2 // 2):
                nc.tensor.matmul(out=pf[:],
                                 lhsT=w2[:, 2 * j:2 * j + 2, m * 128:(m + 1) * 128],
                                 rhs=hb3[:, 2 * j:2 * j + 2, :],
                                 start=(j == 0), stop=(j == NK2 // 2 - 1),
                                 perf_mode=DR)
            # x += pf/4096 + b2 in one fused DVE op
            nc.vector.affine_then_add(out=x[m], in0=pf[:], in1=x[m],
                                      scale=1.0 / 4096, bias=b2[:, m:m + 1])

        ln_inplace(ln2gr, ln2b)
        tap(f"ln2_{l}")

    # final LN + AllGather + lm_head (token-major output via swapped operands)
    ln_inplace(lnf_gr, lnf_b)
    tap("lnf")
    xbg = allgather_x(n_layers, fp8=False)
    for vs in range(NVS):
        wlm = wpool.tile([128, ND, 500], f16, name="wlm", tag="wlm", bufs=3)
        nc.sync.dma_start(wlm[:], E["embT_ap"][:, vs * 500:(vs + 1) * 500]
                          .rearrange("(k p) n -> p k n", p=128))
        for th in range(2):
            lsb4 = act.tile([128, 4 * 500], f16, name="lsb4", tag="lsb4")
            for tt in range(4):
                t = th * 4 + tt
                pl = ps.tile([128, 500], f32, name="pl", tag="mm")
                for k in range(ND):
                    nc.tensor.matmul(out=pl[:], lhsT=xbg[k][:, t * 128:(t + 1) * 128],
                                     rhs=wlm[:, k, :], start=(k == 0), stop=(k == ND - 1))
                sl = lsb4[:, tt * 500:(tt + 1) * 500]
                if t % 2 == 0:
                    nc.scalar.copy(out=sl, in_=pl[:])
                else:
                    nc.vector.tensor_copy(out=sl, in_=pl[:])
            nc.sync.dma_start(
                E["out_ap"][th * 512:(th + 1) * 512, vs * 500:(vs + 1) * 500]
                .rearrange("(t p) v -> p t v", p=128),
                lsb4[:].rearrange("p (t v) -> p t v", t=4))


# ----------------------------------------------------------------- host prep
def _posenc():
    import math
    pos = np.arange(S, dtype=np.float32)[:, None]
    div = np.exp(np.arange(0, D, 2, dtype=np.float32) * (-math.log(10000.0) / D))
    pe = np.zeros((S, D), np.float32)
    pe[:, 0::2] = np.sin(pos * div)
    pe[:, 1::2] = np.cos(pos * div)
    return pe


def _masks_for(rank, not_pad):
    # additive masks: 0 where visible, -60000 where masked (exp -> 0)
    wins = ((rank + 1) * W, (8 - rank) * W)
    out = np.zeros((NMASK, 128, 512), np.float32)
    i = 0
    for slot in (0, 1):
        win = wins[slot]
        for sp, t in SLOT_KT[slot]:
            q = sp * 512 + np.arange(512)[None, :]
            k = t * 128 + np.arange(128)[:, None]
            rel = q - k
            valid = (rel >= 0) & (rel < win) & not_pad[t * 128:(t + 1) * 128, None]
            out[i] = np.where(valid, 0.0, -60000.0)
            i += 1
    return out.astype(np.float16)


def _f16(a):
    return np.ascontiguousarray(a).astype(np.float16)


def _f32c(a):
    return np.ascontiguousarray(np.asarray(a, np.float32))


def _prep_core(inputs, core, n_layers):
    g, r = divmod(core, TP)
    hA, hB = r, 7 - r
    ids_full = np.asarray(inputs["input_ids"][g]).astype(np.int32)
    ids = ids_full[r * TS:(r + 1) * TS]
    emb = _f32c(inputs["emb"])
    Wqkv = _f32c(inputs["Wqkv"])
    bqkv = _f32c(inputs["bqkv"])
    Wo = _f32c(inputs["Wo"])
    bo = _f32c(inputs["bo"])
    W1 = _f32c(inputs["W1"])
    b1 = _f32c(inputs["b1"])
    W2 = _f32c(inputs["W2"])
    b2 = _f32c(inputs["b2"])

    def hcols(W_, base, h):
        return W_[:, :, base + h * HD:base + (h + 1) * HD]

    # wqkvT: [L, 512(din), 384] cols [qA qB kA kB vA vB]; q part pre-scaled 1/8
    WqkvT = Wqkv.transpose(0, 2, 1)  # [L, D(in), 3D(out)]
    wq = np.concatenate([hcols(WqkvT, 0, hA), hcols(WqkvT, 0, hB)], axis=2) / 8.0
    wk = np.concatenate([hcols(WqkvT, D, hA), hcols(WqkvT, D, hB)], axis=2)
    wv = np.concatenate([hcols(WqkvT, 2 * D, hA), hcols(WqkvT, 2 * D, hB)], axis=2)
    wqkvT = _f16(np.concatenate([wq, wk, wv], axis=2))

    def hseg(v, base, h):
        return v[:, base + h * HD:base + (h + 1) * HD]

    bq = np.concatenate([hseg(bqkv, 0, hA), hseg(bqkv, 0, hB)], axis=1) / 8.0
    bk = np.concatenate([hseg(bqkv, D, hA), hseg(bqkv, D, hB)], axis=1)
    bqk = np.ascontiguousarray(
        np.concatenate([bq, bk], axis=1)[:, :, None].astype(np.float32))
    bv = np.concatenate([hseg(bqkv, 2 * D, hA), hseg(bqkv, 2 * D, hB)], axis=1)
    vbias = np.ascontiguousarray(
        np.broadcast_to(bv[:, None, :], (bv.shape[0], 128, 128)).astype(np.float32))

    # woT rows for this rank's two heads (a-row block r*128:(r+1)*128 after
    # the head-major permutation used on-core)
    WoT = Wo.transpose(0, 2, 1)  # [L, D(in, head-major), D(out)]
    wo_rows = np.concatenate(
        [WoT[:, hA * HD:(hA + 1) * HD, :], WoT[:, hB * HD:(hB + 1) * HD, :]],
        axis=1)  # [L, 128, D]
    woT = _f16(wo_rows)

    # full FFN weights (replicated): W1 [L, 4096, 512] -> w1T [L, 512, 4096]
    w1T = np.ascontiguousarray(W1.transpose(0, 2, 1)).astype(np.float32)
    w2T = np.ascontiguousarray(W2.transpose(0, 2, 1)).astype(np.float32)
    # b1 u-half pre-scaled 16x (usb is held at 16x true scale for fp8 range)
    b1 = b1.copy()
    b1[:, :FF] *= 16.0

    not_pad = ids_full != 0
    x0 = emb[ids] + _posenc()[r * TS:(r + 1) * TS]   # [TS, D] fp32
    vals = {
        "x0": _f16(x0.T),
        "embT_lm": _f16(emb[r * VS:(r + 1) * VS].T),
        "wqkvT": wqkvT[:n_layers],
        "bqk": bqk[:n_layers],
        "vbias": vbias[:n_layers],
        "woT": woT[:n_layers],
        "bo": np.ascontiguousarray(bo[:, :, None])[:n_layers],
        "w1T": w1T[:n_layers],
        "b1": np.ascontiguousarray(b1[:, :, None])[:n_layers],
        "w2T": w2T[:n_layers],
        "b2": np.ascontiguousarray(b2[:, :, None])[:n_layers],
        "ln1g": np.ascontiguousarray(_f32c(inputs["ln1_g"])[:, :, None])[:n_layers],
        "ln1b": np.ascontiguousarray(_f32c(inputs["ln1_b"])[:, :, None])[:n_layers],
        "ln2g": np.ascontiguousarray(_f32c(inputs["ln2_g"])[:, :, None])[:n_layers],
        "ln2b": np.ascontiguousarray(_f32c(inputs["ln2_b"])[:, :, None])[:n_layers],
        "lnfg": np.ascontiguousarray(_f32c(inputs["lnf_g"])[:, None]),
        "ln1g_row": np.ascontiguousarray(_f32c(inputs["ln1_g"])[:, None, :])[:n_layers],
        "ln2g_row": np.ascontiguousarray(_f32c(inputs["ln2_g"])[:, None, :])[:n_layers],
        "lnfg_row": np.ascontiguousarray(_f32c(inputs["lnf_g"])[None, :]),
        "lnfb": np.ascontiguousarray(_f32c(inputs["lnf_b"])[:, None]),
        "mask": _masks_for(r, not_pad),
    }
    spec16, spec32, spec8 = _pack_spec(n_layers)
    for nm, shp in spec16 + spec32 + spec8:
        assert tuple(vals[nm].shape) == tuple(shp), (nm, vals[nm].shape, shp)
    f8np = mybir.dt.np(f8)
    pk16 = np.concatenate(
        [np.ascontiguousarray(vals[nm]).astype(np.float16).ravel()
         for nm, _ in spec16])
    pk32 = np.concatenate(
        [np.ascontiguousarray(vals[nm]).astype(np.float32).ravel()
         for nm, _ in spec32])
    segs8 = [np.clip(np.ascontiguousarray(vals[nm]) * 256.0, -240.0, 240.0)
             .astype(f8np).ravel() for nm, _ in spec8 if vals[nm].size]
    pk8 = np.concatenate(segs8) if segs8 else np.zeros(1, f8np)
    return {"pk16": pk16, "pk32": pk32, "pk8": pk8}


def kernel(**inputs):
    global LAST_RESULTS
    n_layers = int(os.environ.get("KERNEL_LAYERS", L))
    taps = tuple(t for t in os.environ.get("KERNEL_TAPS", "").split(",") if t)
    key = (n_layers, taps)
    if key not in _CACHE:
        _CACHE[key] = build_nc(n_layers, taps)
    nc = _CACHE[key]
    in_maps = [_prep_core(inputs, c, n_layers) for c in range(NC)]
    res = bass_utils.run_bass_kernel_spmd(nc, in_maps, core_ids=list(range(NC)))
    LAST_RESULTS = res
    out = np.empty((B, S, V), np.float32)
    for g in range(B):
        for r in range(TP):
            out[g][:, r * VS:(r + 1) * VS] = res.results[g * TP + r][
                "logits"].astype(np.float32)
    return out



# revision 61
# speedup vs baseline: 7.9256x; 7.9256x over previous
"""Trainium2 Bass kernel for nn_DecoderModel_12352325943321.

6-layer post-LN decoder, SwiGLU FFN, per-head staggered windowed causal
attention (head h window (h+1)*64), tied-embedding lm_head.
B=2, S=1024, D=512, H=8, L=6, V=32000.

Sharding (8 NeuronCores): DP-2 over batch x sequence-sharded TP-4 within
each group (Korthikanti-style):
  - residual stream x lives sequence-sharded: rank r of a group owns tokens
    [r*256, (r+1)*256) of its batch, feature-major fp16 [512, 256].
  - per layer: AllGather x (1 collective) -> attention for the rank's 2
    heads (r, 7-r) over the full sequence -> partial Wo over the rank's
    head columns -> ReduceScatter back to sequence shards (1 collective)
    -> LN1, full-width SwiGLU FFN (replicated weights, local tokens), LN2
    all local.  2 collectives/layer instead of AG+AllReduce.
  - lm_head: vocab-sharded (rank owns rows [r*8000, (r+1)*8000)) after a
    final AllGather of the lnf output; host concat.

On-core layout: fp16 activations feature-major [D(part), T(free)], fp32
PSUM accumulation, weights host-pretransposed [K, M] (fp16 except the FFN
weights, which are fp8 e4m3 scaled 256x with the unscale folded into the
existing activation/bias ops, and consumed by DoubleRow fp8 matmuls over
k-tile pairs).  Attention masks are additive (0 / -60000) and preloaded
into PSUM by an identity matmul so the score matmul accumulates onto them;
the two head slots' (score, exp, PV) pipelines are interleaved with each
PV deferred two slots so the in-order PE queue never stalls on exp.
LayerNorm reduces mean/sumsq with a ones-column matmul, takes rstd via
exp(-0.5*ln(var+eps)) (same act table as attention's Exp -> no table
reloads), and broadcasts [rstd | -mu*rstd] with gamma folded in via a
1-contraction PE matmul; the apply and both residual adds use the fused
DVE affine_then_add.  All inputs are packed into three flat tensors (fp16/
fp32/fp8) because the exec stack pays a per-input-tensor cost every call.
Per-layer weight DMAs are prefetched at the top of the layer on the sync
queue; collective-adjacent DMAs ride the scalar queue so neither blocks
the other.  The SPMD graph is rank-independent: all per-rank differences
(head windows, weight slices, masks, token shards) live in input data, so
every core executes the same NEFF.
"""
import os
from contextlib import ExitStack
import numpy as np
import ml_dtypes

import concourse.bass as bass
import concourse.bacc as bacc
import concourse.tile as tile
import concourse.mybir as mybir
from concourse import bass_utils
from concourse.masks import make_identity

f32 = mybir.dt.float32
f16 = mybir.dt.float16
f8 = mybir.dt.float8e4
bf16 = mybir.dt.bfloat16
i32 = mybir.dt.int32
AF = mybir.ActivationFunctionType
ALU = mybir.AluOpType

B, S, D, H, L, V, W, FF = 2, 1024, 512, 8, 6, 32000, 64, 2048
HD = D // H          # 64
TP = 4               # tensor-parallel ranks per group
NC = 8
TS = S // TP         # 256 tokens owned per rank
VS = V // TP         # 8000 vocab rows per rank
NT = S // 128        # 8 token tiles (full seq)
ND = D // 128        # 4 feature tiles
NSP = S // 512       # 2 token spans of 512
NH = 2 * FF // 128   # 32 hidden tiles (u+g)
NK2 = FF // 128      # 16 k-tiles for W2
NVS = VS // 500      # 16 vocab subtiles
EPS = 1e-5

# fixed attention (span, ktile) schedule per head slot; A: win<=256, B: <=512
SLOT_KT = {
    0: [(0, t) for t in range(0, 4)] + [(1, t) for t in range(2, 8)],
    1: [(0, t) for t in range(0, 4)] + [(1, t) for t in range(0, 8)],
}
NMASK = len(SLOT_KT[0]) + len(SLOT_KT[1])  # 22

_CACHE = {}
LAST_RESULTS = None
GP = False  # offload mask/residual/hb/t1 elementwise to gpsimd (else vector)
NOCOLL = False  # timing experiment: replace collectives with local DMA copies


# Input packing: the execution stack pays a per-ExternalInput-tensor cost on
# EVERY exec (~35-40us each through the PJRT/axon path), so all inputs are
# packed into two flat tensors (one per dtype) and carved up with AP views.
def _pack_spec(n_layers):
    spec16 = [
        ("x0", (D, TS)), ("embT_lm", (D, VS)),
        ("woT", (n_layers, 128, D)), ("mask", (NMASK, 128, 512)),
        ("wqkvT", (n_layers, D, 384)), ("ln1g_row", (n_layers, 1, D)),
        ("ln2g_row", (n_layers, 1, D)), ("lnfg_row", (1, D)),
    ]
    spec32 = [
        ("bqk", (n_layers, 256, 1)), ("vbias", (n_layers, 128, 128)),
        ("bo", (n_layers, D, 1)), ("b1", (n_layers, 2 * FF, 1)),
        ("b2", (n_layers, D, 1)), ("ln1g", (n_layers, D, 1)),
        ("ln1b", (n_layers, D, 1)), ("ln2g", (n_layers, D, 1)),
        ("ln2b", (n_layers, D, 1)), ("lnfg", (D, 1)), ("lnfb", (D, 1)),
    ]
    # fp8 FFN weights, host-scaled by 256 (e4m3, unscale folded downstream)
    spec8 = [
        ("w1T", (n_layers, D, 2 * FF)), ("w2T", (n_layers, FF, D)),
    ]
    return spec16, spec32, spec8


def _carve(base_ap, spec):
    aps = {}
    off = 0
    for nm, shp in spec:
        n = int(np.prod(shp))
        if n == 0:
            aps[nm] = None
            continue
        ap = base_ap[off:off + n]
        if len(shp) > 1:
            letters = list("abcde"[:len(shp)])
            pat = f"({' '.join(letters)}) -> {' '.join(letters)}"
            ap = ap.rearrange(pat, **{letters[i]: int(shp[i])
                                      for i in range(len(shp) - 1)})
        aps[nm] = ap
        off += n
    return aps


# ----------------------------------------------------------------- build
def build_nc(n_layers=L, taps=()):
    nc = bacc.Bacc("TRN2", target_bir_lowering=False, debug=False,
                   enable_asserts=True, num_devices=NC)

    spec16, spec32, spec8 = _pack_spec(n_layers)
    tot16 = sum(int(np.prod(shp)) for _, shp in spec16)
    tot32 = sum(int(np.prod(shp)) for _, shp in spec32)
    tot8 = sum(int(np.prod(shp)) for _, shp in spec8)
    aps16 = _carve(nc.dram_tensor("pk16", [tot16], f16,
                                  kind="ExternalInput").ap(), spec16)
    aps32 = _carve(nc.dram_tensor("pk32", [tot32], f32,
                                  kind="ExternalInput").ap(), spec32)
    aps8 = _carve(nc.dram_tensor("pk8", [max(tot8, 1)], f8,
                                 kind="ExternalInput").ap(), spec8)

    E = {
        "n_layers": n_layers,
        "groups": [[0, 1, 2, 3], [4, 5, 6, 7]],
        "x0_ap": aps16["x0"],
        "embT_ap": aps16["embT_lm"],
        "wqkv_ap": aps16["wqkvT"],
        "bqk_ap": aps32["bqk"],
        "vbias_ap": aps32["vbias"],
        "wo_ap": aps16["woT"],
        "bo_ap": aps32["bo"],
        "w1_ap": aps8["w1T"],
        "b1_ap": aps32["b1"],
        "w2_ap": aps8["w2T"],
        "b2_ap": aps32["b2"],
        "ln1g_ap": aps32["ln1g"],
        "ln1b_ap": aps32["ln1b"],
        "ln2g_ap": aps32["ln2g"],
        "ln2b_ap": aps32["ln2b"],
        "lnfg_ap": aps32["lnfg"],
        "lnfb_ap": aps32["lnfb"],
        "mask_ap": aps16["mask"],
        "ln1gr_ap": aps16["ln1g_row"],
        "ln2gr_ap": aps16["ln2g_row"],
        "lnfgr_ap": aps16["lnfg_row"],
        "out_ap": nc.dram_tensor("logits", [S, VS], f16, kind="ExternalOutput").ap(),
        "tap_aps": {t: nc.dram_tensor(f"tap_{t}", [D, TS], f32,
                                      kind="ExternalOutput").ap() for t in taps},
        "ag_in": [nc.dram_tensor(f"ag_in{l}", [D, TS],
                                 f8 if l < n_layers else f16,
                                 kind="Internal").ap()
                  for l in range(n_layers + 1)],
        "ag_out": [nc.dram_tensor(f"ag_out{l}", [TP * D, TS],
                                  f8 if l < n_layers else f16,
                                  kind="Internal").ap()
                   for l in range(n_layers + 1)],
        "rs_in": [nc.dram_tensor(f"rs_in{l}", [TP * D, TS], f16,
                                 kind="Internal").ap()
                  for l in range(n_layers)],
        "rs_out": [nc.dram_tensor(f"rs_out{l}", [D, TS], f16,
                                  kind="Internal").ap()
                   for l in range(n_layers)],
    }

    with tile.TileContext(nc) as tc:
        _emit(tc, E)
    nc.compile()
    return nc


def _emit(tc, E):
    with ExitStack() as _ctx:
        _emit_body(tc, E, _ctx)


def _emit_body(tc, E, ctx):
    nc = tc.nc
    n_layers = E["n_layers"]
    taps = E["tap_aps"]

    const = ctx.enter_context(tc.tile_pool(name="const", bufs=1))
    resid = ctx.enter_context(tc.tile_pool(name="resid", bufs=1))
    wpool = ctx.enter_context(tc.tile_pool(name="wpool", bufs=2))
    act = ctx.enter_context(tc.tile_pool(name="act", bufs=2))   # small transients
    big = ctx.enter_context(tc.tile_pool(name="big", bufs=1))   # per-layer tensors
    lnp = ctx.enter_context(tc.tile_pool(name="lnp", bufs=1))
    ps = ctx.enter_context(tc.tile_pool(name="ps", bufs=2, space="PSUM"))
    ps_att = ctx.enter_context(tc.tile_pool(name="ps_att", bufs=3, space="PSUM"))
    ps_pv = ctx.enter_context(tc.tile_pool(name="ps_pv", bufs=1, space="PSUM"))
    ps_sm = ctx.enter_context(tc.tile_pool(name="ps_sm", bufs=1, space="PSUM"))

    # ---------------- constants
    ident16 = const.tile([128, 128], f16, name="ident16")
    make_identity(nc, ident16[:])
    ones_col = const.tile([128, 1], f32, name="ones_col")
    nc.any.memset(ones_col[:], 1.0)
    ones_row = const.tile([1, 128], f16, name="ones_row")
    nc.any.memset(ones_row[:], 1.0)
    eps1 = const.tile([1, 1], f32, name="eps1")
    nc.any.memset(eps1[:], EPS)
    masks = [const.tile([128, 512], f16, name=f"mask{i}") for i in range(NMASK)]
    for i in range(NMASK):
        nc.scalar.dma_start(masks[i][:], E["mask_ap"][i])

    def load_dvec(ap_2d, pool, name, width=ND):
        t = pool.tile([128, width], f32, name=name, tag=name)
        nc.sync.dma_start(t[:], ap_2d.rearrange("(d p) one -> p (d one)", p=128))
        return t

    lnf_gr = const.tile([1, D], f16, name="lnf_gr")
    nc.sync.dma_start(lnf_gr[:], E["lnfgr_ap"][:])
    lnf_b = load_dvec(E["lnfb_ap"], const, "lnf_b")

    # residual stream: fp16 feature-major, local 256 tokens, in ONE tile so
    # adjacent d-tiles can be pair-sliced for DoubleRow fp8 matmuls
    # (embedding gather + posenc precomputed on host as x0 [D, TS] fp16)
    x_all = resid.tile([128, ND * TS], f16, name="x_all")
    x = [x_all[:, d * TS:(d + 1) * TS] for d in range(ND)]
    for d in range(ND):
        nc.sync.dma_start(x[d], E["x0_ap"][d * 128:(d + 1) * 128, :])

    # ---------------- local feature-major LayerNorm (updates x in place)
    # grow: gamma as an f16 row [1, D]; bvec: beta [128, ND] f32
    def ln_inplace(grow, bvec):
        fold2 = lnp.tile([128, 2 * TS], f32, name="fold2", tag="fold2")
        fold = fold2[:, 0:TS]
        sqf = fold2[:, TS:2 * TS]
        f01 = lnp.tile([128, TS], f32, name="f01", tag="f01")
        nc.vector.tensor_tensor(out=f01[:], in0=x[0][:], in1=x[1][:], op=ALU.add)
        nc.vector.tensor_tensor(out=fold, in0=x[2][:], in1=x[3][:], op=ALU.add)
        nc.vector.tensor_tensor(out=fold, in0=fold, in1=f01[:], op=ALU.add)
        # squares on the Act engine (Square shares the exp/ln table -> no
        # table reload); sums of squares folded on DVE
        nc.scalar.activation(sqf, x[0][:], AF.Square)
        sq = lnp.tile([128, TS], f32, name="sq", tag="sq")
        sq2 = lnp.tile([128, TS], f32, name="sq2", tag="sq2")
        nc.scalar.activation(sq[:], x[1][:], AF.Square)
        nc.vector.tensor_tensor(out=sqf, in0=sqf, in1=sq[:], op=ALU.add)
        nc.scalar.activation(sq2[:], x[2][:], AF.Square)
        nc.scalar.activation(sq[:], x[3][:], AF.Square)
        nc.vector.tensor_tensor(out=sq2[:], in0=sq2[:], in1=sq[:], op=ALU.add)
        nc.vector.tensor_tensor(out=sqf, in0=sqf, in1=sq2[:], op=ALU.add)
        st = ps_sm.tile([1, 2 * TS], f32, name="st", tag="sm")
        nc.tensor.matmul(out=st[:], lhsT=ones_col[:], rhs=fold2[:],
                         start=True, stop=True)
        # negated mean so the apply tail is x*(rstd*g) + (-m*rstd*g) + b
        mn = lnp.tile([1, 2 * TS], f32, name="mn", tag="mn")
        nc.vector.tensor_scalar(out=mn[:, 0:TS], in0=st[:, 0:TS],
                                scalar1=-1.0 / D, scalar2=None, op0=ALU.mult)
        nc.vector.tensor_scalar(out=mn[:, TS:2 * TS], in0=st[:, TS:2 * TS],
                                scalar1=1.0 / D, scalar2=None, op0=ALU.mult)
        mean_neg = mn[:, 0:TS]
        var = mn[:, TS:2 * TS]
        msq = lnp.tile([1, TS], f32, name="msq", tag="msq")
        nc.vector.tensor_tensor(out=msq[:], in0=mean_neg, in1=mean_neg,
                                op=ALU.mult)
        nc.vector.tensor_tensor(out=var, in0=var, in1=msq[:], op=ALU.subtract)
        # rstd = (var+eps)^-0.5 via exp(-0.5*ln(var+eps)): Ln and Exp live in
        # the same activation table as attention's Exp -> no table reloads
        lnv = lnp.tile([1, TS], f32, name="lnv", tag="lnv")
        nc.scalar.activation(lnv[:], var, AF.Ln, bias=eps1[:], scale=1.0)
        rstd = lnp.tile([1, TS], f32, name="rstd", tag="rstd")
        nc.scalar.activation(rstd[:], lnv[:], AF.Exp, scale=-0.5)
        rm16 = lnp.tile([1, 2 * TS], f16, name="rm16", tag="rm16")
        nc.vector.tensor_copy(out=rm16[:, 0:TS], in_=rstd[:])
        nc.vector.tensor_tensor(out=rm16[:, TS:2 * TS], in0=mean_neg,
                                in1=rstd[:], op=ALU.mult)
        # per-d-tile broadcast with gamma folded in: a 1-contraction PE
        # matmul gives bcg = gamma_d (x) [rstd | -m*rstd]; the apply is then
        # two DVE ops: t1 = x*bcg_lo; x = (t1 + beta) + bcg_hi
        for d in range(ND):
            bcg = ps.tile([128, 2 * TS], f32, name="bcg", tag="mm")
            nc.tensor.matmul(out=bcg[:], lhsT=grow[:, d * 128:(d + 1) * 128],
                             rhs=rm16[:], start=True, stop=True)
            t1 = lnp.tile([128, TS], f16, name="t1", tag="t1")
            nc.vector.tensor_tensor(out=t1[:], in0=x[d][:], in1=bcg[:, 0:TS],
                                    op=ALU.mult)
            nc.vector.affine_then_add(out=x[d], in0=t1[:],
                                      in1=bcg[:, TS:2 * TS], scale=1.0,
                                      bias=bvec[:, d:d + 1])

    def tap(name):
        if name not in taps:
            return
        for d in range(ND):
            tf = lnp.tile([128, TS], f32, name="tapf", tag="t1")
            nc.vector.tensor_copy(out=tf[:], in_=x[d][:])
            nc.sync.dma_start(taps[name][d * 128:(d + 1) * 128, :], tf[:])

    def allgather_x(l, fp8):
        # all DMAs here ride the scalar queue: they gate (or are gated by) the
        # collective, and must not sit behind weight prefetches on sync
        if fp8:
            for d in range(ND):
                x8 = act.tile([128, TS], f8, name="x8", tag="x8")
                nc.vector.tensor_copy(out=x8[:], in_=x[d][:])
                nc.scalar.dma_start(E["ag_in"][l][d * 128:(d + 1) * 128, :], x8[:])
        else:
            for d in range(ND):
                nc.scalar.dma_start(E["ag_in"][l][d * 128:(d + 1) * 128, :], x[d][:])
        if NOCOLL:
            for rr in range(TP):
                nc.scalar.dma_start(E["ag_out"][l][rr * D:(rr + 1) * D, :],
                                    E["ag_in"][l][:])
        else:
            nc.gpsimd.collective_compute(
                "AllGather", ALU.bypass, replica_groups=E["groups"],
                ins=[E["ag_in"][l][:].opt()], outs=[E["ag_out"][l][:].opt()])
        xg = [big.tile([128, S], f16, name=f"xg{d}", tag=f"xg{d}")
              for d in range(ND)]
        src = E["ag_out"][l].rearrange("(r f) t -> f r t", r=TP)
        for d in range(ND):
            if fp8:
                xg8 = big.tile([128, S], f8, name=f"xg8_{d}", tag=f"xg8_{d}")
                nc.scalar.dma_start(
                    xg8[:].rearrange("p (r t) -> p r t", r=TP),
                    src[d * 128:(d + 1) * 128])
                nc.vector.tensor_copy(out=xg[d][:], in_=xg8[:])
            else:
                nc.scalar.dma_start(
                    xg[d][:].rearrange("p (r t) -> p r t", r=TP),
                    src[d * 128:(d + 1) * 128])
        return xg

    # ---------------- layers
    for l in range(n_layers):
        # prefetch ALL layer params first, on the sync queue (keeps the queue
        # free of collective-dependent loads so weight DMAs never stall)
        wqkv = wpool.tile([128, ND, 384], f16, name="wqkv", tag="wqkv")
        nc.sync.dma_start(wqkv[:], E["wqkv_ap"][l].rearrange("(k p) m -> p k m", p=128))
        bqk = wpool.tile([128, 2], f32, name="bqk", tag="bqk")
        nc.sync.dma_start(bqk[:], E["bqk_ap"][l].rearrange("(a p) one -> p (a one)", p=128))
        vbias = wpool.tile([128, 128], f32, name="vbias", tag="vbias")
        nc.sync.dma_start(vbias[:], E["vbias_ap"][l])
        wo = wpool.tile([128, D], f16, name="wo", tag="wo")
        nc.sync.dma_start(wo[:], E["wo_ap"][l])
        bo_t = load_dvec(E["bo_ap"][l], wpool, "bo_t")
        ln1gr = wpool.tile([1, D], f16, name="ln1gr", tag="ln1gr")
        nc.sync.dma_start(ln1gr[:], E["ln1gr_ap"][l])
        ln1b = load_dvec(E["ln1b_ap"][l], wpool, "ln1b")
        w1 = wpool.tile([128, ND, 2 * FF], f8, name="w1", tag="w1", bufs=1)
        nc.sync.dma_start(w1[:], E["w1_ap"][l].rearrange("(k p) m -> p k m", p=128))
        b1 = load_dvec(E["b1_ap"][l], wpool, "b1", width=NH)
        w2 = wpool.tile([128, NK2, D], f8, name="w2", tag="w2", bufs=1)
        nc.sync.dma_start(w2[:], E["w2_ap"][l].rearrange("(k p) m -> p k m", p=128))
        b2 = load_dvec(E["b2_ap"][l], wpool, "b2")
        ln2gr = wpool.tile([1, D], f16, name="ln2gr", tag="ln2gr")
        nc.sync.dma_start(ln2gr[:], E["ln2gr_ap"][l])
        ln2b = load_dvec(E["ln2b_ap"][l], wpool, "ln2b")

        xg = allgather_x(l, fp8=True)
        DR = mybir.MatmulPerfMode.DoubleRow

        # q, k feature-major [128, S] fp16 (rows: slotA 0:64, slotB 64:128)
        q_sb = big.tile([128, S], f16, name="q_sb", tag="q_sb")
        k_sb = big.tile([128, S], f16, name="k_sb", tag="k_sb")
        for mi, dest in ((0, q_sb), (1, k_sb)):
            for sp in range(NSP):
                sl = slice(sp * 512, (sp + 1) * 512)
                pm = ps.tile([128, 512], f32, name="pm_qk", tag="mm")
                for k in range(ND):
                    nc.tensor.matmul(out=pm[:],
                                     lhsT=wqkv[:, k, mi * 128:(mi + 1) * 128],
                                     rhs=xg[k][:, sl],
                                     start=(k == 0), stop=(k == ND - 1))
                nc.scalar.activation(dest[:, sl], pm[:], AF.Identity,
                                     bias=bqk[:, mi:mi + 1])

        # v token-major per tok-tile: [128, 130] = [vA(64) | 1 | vB(64) | 1]
        vts = []
        for t in range(NT):
            pv = ps.tile([128, 128], f32, name="pv_v", tag="mm")
            for k in range(ND):
                nc.tensor.matmul(out=pv[:], lhsT=xg[k][:, t * 128:(t + 1) * 128],
                                 rhs=wqkv[:, k, 256:384],
                                 start=(k == 0), stop=(k == ND - 1))
            vsb = big.tile([128, 130], f16, name=f"v65_{t}", tag=f"v65_{t}")
            nc.vector.tensor_tensor(out=vsb[:, 0:64], in0=pv[:, 0:64],
                                    in1=vbias[:, 0:64], op=ALU.add)
            nc.vector.tensor_tensor(out=vsb[:, 65:129], in0=pv[:, 64:128],
                                    in1=vbias[:, 64:128], op=ALU.add)
            nc.any.memset(vsb[:, 64:65], 1.0)
            nc.any.memset(vsb[:, 129:130], 1.0)
            vts.append(vsb)

        # attention: fixed 22 k-tile schedule; 0/1 masks multiply exp(scores).
        # The two head slots' accumulation groups are interleaved and each PV
        # matmul is deferred two schedule slots so the in-order PE queue does
        # other tiles' score matmuls while exp+mask of this tile complete.
        a_sb = big.tile([128, S], f16, name="a_sb", tag="a_sb")
        mask_id = {}
        mi_idx = 0
        for slot in (0, 1):
            for sp, t in SLOT_KT[slot]:
                mask_id[(slot, sp, t)] = mi_idx
                mi_idx += 1

        def attn_span(sp):
            # interleaved tile sequence for the two slots on this span
            seqs = []
            for slot in (0, 1):
                kts = [t for s_, t in SLOT_KT[slot] if s_ == sp]
                seqs.append([(slot, t, i, len(kts)) for i, t in enumerate(kts)])
            order = []
            i = j = 0
            while i < len(seqs[0]) or j < len(seqs[1]):
                if i < len(seqs[0]):
                    order.append(seqs[0][i]); i += 1
                if j < len(seqs[1]):
                    order.append(seqs[1][j]); j += 1
            qsl = slice(sp * 512, (sp + 1) * 512)
            pvp = {s: ps_pv.tile([65, 512], f32, name=f"pvp{s}", tag=f"pvp{s}")
                   for s in (0, 1)}
            pending = []

            def flush_one():
                slot, t, i, n, p_sb = pending.pop(0)
                nc.tensor.matmul(out=pvp[slot][:],
                                 lhsT=vts[t][:, slot * 65:slot * 65 + 65],
                                 rhs=p_sb[:], start=(i == 0), stop=(i == n - 1))

            for slot, t, i, n in order:
                rows = slice(slot * 64, slot * 64 + 64)
                scp = ps_att.tile([128, 512], f32, name="scp", tag="scp")
                # additive mask (0 / -60000) preloaded into PSUM by an
                # identity matmul; the score matmul accumulates onto it
                nc.tensor.matmul(out=scp[:], lhsT=ident16[:],
                                 rhs=masks[mask_id[(slot, sp, t)]][:],
                                 start=True, stop=False)
                nc.tensor.matmul(out=scp[:], lhsT=k_sb[rows, t * 128:(t + 1) * 128],
                                 rhs=q_sb[rows, qsl], start=False, stop=True)
                p_sb = act.tile([128, 512], f16, name="p_sb", tag="p_sb", bufs=4)
                nc.scalar.activation(p_sb[:], scp[:], AF.Exp)
                pending.append((slot, t, i, n, p_sb))
                if len(pending) > 2:
                    flush_one()
            while pending:
                flush_one()
            for slot in (0, 1):
                rows = slice(slot * 64, slot * 64 + 64)
                den = act.tile([1, 512], f32, name="den", tag="den")
                nc.vector.reciprocal(out=den[:], in_=pvp[slot][64:65, :])
                den_b = act.tile([64, 512], f32, name="den_b", tag="den_b")
                nc.gpsimd.partition_broadcast(den_b[:], den[:])
                nc.vector.tensor_tensor(out=a_sb[rows, qsl], in0=pvp[slot][0:64, :],
                                        in1=den_b[:], op=ALU.mult)

        for sp in range(NSP):
            attn_span(sp)

        # partial Wo over this rank's head columns -> ReduceScatter
        for m in range(ND):
            for sp in range(NSP):
                sl = slice(sp * 512, (sp + 1) * 512)
                pm = ps.tile([128, 512], f32, name="pm_wo", tag="mm")
                nc.tensor.matmul(out=pm[:], lhsT=wo[:, m * 128:(m + 1) * 128],
                                 rhs=a_sb[:, sl], start=True, stop=True)
                fsb = act.tile([128, 512], f16, name="fsb", tag="fsb")
                nc.vector.tensor_copy(out=fsb[:], in_=pm[:])
                dst = E["rs_in"][l].rearrange("(dest f) t -> f dest t", dest=TP)
                # sync queue: layer-l weight prefetches have drained by now,
                # and keeping these off the scalar queue frees the Act engine
                nc.sync.dma_start(
                    dst[m * 128:(m + 1) * 128, 2 * sp:2 * sp + 2],
                    fsb[:].rearrange("p (h t) -> p h t", h=2))
        if NOCOLL:
            nc.scalar.dma_start(E["rs_out"][l][:], E["rs_in"][l][0:D, :])
        else:
            nc.gpsimd.collective_compute(
                "ReduceScatter", ALU.add, replica_groups=E["groups"],
                ins=[E["rs_in"][l][:].opt()], outs=[E["rs_out"][l][:].opt()])

        rt4 = act.tile([128, ND * TS], f16, name="rt4", tag="rt4")
        nc.sync.dma_start(rt4[:].rearrange("p (d t) -> p d t", d=ND),
                          E["rs_out"][l].rearrange("(d f) t -> f d t", d=ND))
        for d in range(ND):
            # x += rt4_d + bo in one fused DVE op
            nc.vector.affine_then_add(out=x[d], in0=rt4[:, d * TS:(d + 1) * TS],
                                      in1=x[d], scale=1.0,
                                      bias=bo_t[:, d:d + 1])

        tap(f"res1_{l}")
        ln_inplace(ln1gr, ln1b)
        tap(f"ln1_{l}")

        # FFN: full hidden width on local tokens, hidden-major.  fp8x256
        # weights, fp8 activations (x re-quantized post-LN1), DoubleRow over
        # k-pairs.  Scale ledger: pu/pg = 256x; usb = 16x (scale 1/16, b1u
        # host-prescaled 16x); gsb = true (scale 1/256); hb = 16x fp8;
        # pf = 256*16x -> ot = pf/4096 + b2.
        x8f = big.tile([128, ND * TS], f8, name="x8f", tag="x8f")
        for d in range(ND):
            nc.vector.tensor_copy(out=x8f[:, d * TS:(d + 1) * TS], in_=x[d])
        x83 = x8f[:].rearrange("p (d t) -> p d t", d=ND)
        hb_all = big.tile([128, NK2 * TS], f8, name="hb_all")
        hb3 = hb_all[:].rearrange("p (k t) -> p k t", k=NK2)
        for m in range(NK2):
            pu = ps.tile([128, TS], f32, name="pu", tag="mm")
            for j in range(ND // 2):
                nc.tensor.matmul(out=pu[:],
                                 lhsT=w1[:, 2 * j:2 * j + 2, m * 128:(m + 1) * 128],
                                 rhs=x83[:, 2 * j:2 * j + 2, :],
                                 start=(j == 0), stop=(j == ND // 2 - 1),
                                 perf_mode=DR)
            pg = ps.tile([128, TS], f32, name="pg", tag="mm")
            for j in range(ND // 2):
                nc.tensor.matmul(out=pg[:],
                                 lhsT=w1[:, 2 * j:2 * j + 2,
                                          FF + m * 128:FF + (m + 1) * 128],
                                 rhs=x83[:, 2 * j:2 * j + 2, :],
                                 start=(j == 0), stop=(j == ND // 2 - 1),
                                 perf_mode=DR)
            usb = act.tile([128, TS], f16, name="usb", tag="usb")
            if m % 2 == 0:
                nc.scalar.activation(usb[:], pu[:], AF.Identity,
                                     bias=b1[:, m:m + 1], scale=1.0 / 16)
            else:
                nc.vector.tensor_scalar(out=usb[:], in0=pu[:], scalar1=1.0 / 16,
                                        scalar2=b1[:, m:m + 1], op0=ALU.mult,
                                        op1=ALU.add)
            gsb = act.tile([128, TS], f16, name="gsb", tag="gsb")
            nc.scalar.activation(gsb[:], pg[:], AF.Silu,
                                 bias=b1[:, NK2 + m:NK2 + m + 1], scale=1.0 / 256)
            (nc.gpsimd if GP else nc.vector).tensor_tensor(
                out=hb3[:, m, :], in0=usb[:], in1=gsb[:], op=ALU.mult)
        for m in range(ND):
            pf = ps.tile([128, TS], f32, name="pf", tag="mm")
            for j in range(NK2 // 2):
                nc.tensor.matmul(out=pf[:],
                                 lhsT=w2[:, 2 * j:2 * j + 2, m * 128:(m + 1) * 128],
                                 rhs=hb3[:, 2 * j:2 * j + 2, :],
                                 start=(j == 0), stop=(j == NK2 // 2 - 1),
                                 perf_mode=DR)
            # x += pf/4096 + b2 in one fused DVE op
            nc.vector.affine_then_add(out=x[m], in0=pf[:], in1=x[m],
                                      scale=1.0 / 4096, bias=b2[:, m:m + 1])

        ln_inplace(ln2gr, ln2b)
        tap(f"ln2_{l}")

    # final LN + AllGather + lm_head (token-major output via swapped operands)
    ln_inplace(lnf_gr, lnf_b)
    tap("lnf")
    xbg = allgather_x(n_layers, fp8=False)
    # 1MB weight chunks and 1MB logits writes (2000B rows) amortize per-DMA
    # fixed costs; the 500-wide matmuls are unchanged (PSUM bank limit)
    for vs in range(NVS // 2):
        wlm = wpool.tile([128, ND, 1000], f16, name="wlm", tag="wlm", bufs=2)
        nc.sync.dma_start(wlm[:], E["embT_ap"][:, vs * 1000:(vs + 1) * 1000]
                          .rearrange("(k p) n -> p k n", p=128))
        for th in range(2):
            lsb4 = act.tile([128, 4 * 1000], f16, name="lsb4", tag="lsb4")
            for tt in range(4):
                t = th * 4 + tt
                for half in range(2):
                    pl = ps.tile([128, 500], f32, name="pl", tag="mm")
                    for k in range(ND):
                        nc.tensor.matmul(
                            out=pl[:], lhsT=xbg[k][:, t * 128:(t + 1) * 128],
                            rhs=wlm[:, k, half * 500:half * 500 + 500],
                            start=(k == 0), stop=(k == ND - 1))
                    sl = lsb4[:, tt * 1000 + half * 500:tt * 1000 + half * 500 + 500]
                    if (t + half) % 2 == 0:
                        nc.scalar.copy(out=sl, in_=pl[:])
                    else:
                        nc.vector.tensor_copy(out=sl, in_=pl[:])
            nc.sync.dma_start(
                E["out_ap"][th * 512:(th + 1) * 512, vs * 1000:(vs + 1) * 1000]
                .rearrange("(t p) v -> p t v", p=128),
                lsb4[:].rearrange("p (t v) -> p t v", t=4))


# ----------------------------------------------------------------- host prep
def _posenc():
    import math
    pos = np.arange(S, dtype=np.float32)[:, None]
    div = np.exp(np.arange(0, D, 2, dtype=np.float32) * (-math.log(10000.0) / D))
    pe = np.zeros((S, D), np.float32)
    pe[:, 0::2] = np.sin(pos * div)
    pe[:, 1::2] = np.cos(pos * div)
    return pe


def _masks_for(rank, not_pad):
    # additive masks: 0 where visible, -60000 where masked (exp -> 0)
    wins = ((rank + 1) * W, (8 - rank) * W)
    out = np.zeros((NMASK, 128, 512), np.float32)
    i = 0
    for slot in (0, 1):
        win = wins[slot]
        for sp, t in SLOT_KT[slot]:
            q = sp * 512 + np.arange(512)[None, :]
            k = t * 128 + np.arange(128)[:, None]
            rel = q - k
            valid = (rel >= 0) & (rel < win) & not_pad[t * 128:(t + 1) * 128, None]
            out[i] = np.where(valid, 0.0, -60000.0)
            i += 1
    return out.astype(np.float16)


def _f16(a):
    return np.ascontiguousarray(a).astype(np.float16)


def _f32c(a):
    return np.ascontiguousarray(np.asarray(a, np.float32))


def _prep_core(inputs, core, n_layers):
    g, r = divmod(core, TP)
    hA, hB = r, 7 - r
    ids_full = np.asarray(inputs["input_ids"][g]).astype(np.int32)
    ids = ids_full[r * TS:(r + 1) * TS]
    emb = _f32c(inputs["emb"])
    Wqkv = _f32c(inputs["Wqkv"])
    bqkv = _f32c(inputs["bqkv"])
    Wo = _f32c(inputs["Wo"])
    bo = _f32c(inputs["bo"])
    W1 = _f32c(inputs["W1"])
    b1 = _f32c(inputs["b1"])
    W2 = _f32c(inputs["W2"])
    b2 = _f32c(inputs["b2"])

    def hcols(W_, base, h):
        return W_[:, :, base + h * HD:base + (h + 1) * HD]

    # wqkvT: [L, 512(din), 384] cols [qA qB kA kB vA vB]; q part pre-scaled 1/8
    WqkvT = Wqkv.transpose(0, 2, 1)  # [L, D(in), 3D(out)]
    wq = np.concatenate([hcols(WqkvT, 0, hA), hcols(WqkvT, 0, hB)], axis=2) / 8.0
    wk = np.concatenate([hcols(WqkvT, D, hA), hcols(WqkvT, D, hB)], axis=2)
    wv = np.concatenate([hcols(WqkvT, 2 * D, hA), hcols(WqkvT, 2 * D, hB)], axis=2)
    wqkvT = _f16(np.concatenate([wq, wk, wv], axis=2))

    def hseg(v, base, h):
        return v[:, base + h * HD:base + (h + 1) * HD]

    bq = np.concatenate([hseg(bqkv, 0, hA), hseg(bqkv, 0, hB)], axis=1) / 8.0
    bk = np.concatenate([hseg(bqkv, D, hA), hseg(bqkv, D, hB)], axis=1)
    bqk = np.ascontiguousarray(
        np.concatenate([bq, bk], axis=1)[:, :, None].astype(np.float32))
    bv = np.concatenate([hseg(bqkv, 2 * D, hA), hseg(bqkv, 2 * D, hB)], axis=1)
    vbias = np.ascontiguousarray(
        np.broadcast_to(bv[:, None, :], (bv.shape[0], 128, 128)).astype(np.float32))

    # woT rows for this rank's two heads (a-row block r*128:(r+1)*128 after
    # the head-major permutation used on-core)
    WoT = Wo.transpose(0, 2, 1)  # [L, D(in, head-major), D(out)]
    wo_rows = np.concatenate(
        [WoT[:, hA * HD:(hA + 1) * HD, :], WoT[:, hB * HD:(hB + 1) * HD, :]],
        axis=1)  # [L, 128, D]
    woT = _f16(wo_rows)

    # full FFN weights (replicated): W1 [L, 4096, 512] -> w1T [L, 512, 4096]
    w1T = np.ascontiguousarray(W1.transpose(0, 2, 1)).astype(np.float32)
    w2T = np.ascontiguousarray(W2.transpose(0, 2, 1)).astype(np.float32)
    # b1 u-half pre-scaled 16x (usb is held at 16x true scale for fp8 range)
    b1 = b1.copy()
    b1[:, :FF] *= 16.0

    not_pad = ids_full != 0
    x0 = emb[ids] + _posenc()[r * TS:(r + 1) * TS]   # [TS, D] fp32
    vals = {
        "x0": _f16(x0.T),
        "embT_lm": _f16(emb[r * VS:(r + 1) * VS].T),
        "wqkvT": wqkvT[:n_layers],
        "bqk": bqk[:n_layers],
        "vbias": vbias[:n_layers],
        "woT": woT[:n_layers],
        "bo": np.ascontiguousarray(bo[:, :, None])[:n_layers],
        "w1T": w1T[:n_layers],
        "b1": np.ascontiguousarray(b1[:, :, None])[:n_layers],
        "w2T": w2T[:n_layers],
        "b2": np.ascontiguousarray(b2[:, :, None])[:n_layers],
        "ln1g": np.ascontiguousarray(_f32c(inputs["ln1_g"])[:, :, None])[:n_layers],
        "ln1b": np.ascontiguousarray(_f32c(inputs["ln1_b"])[:, :, None])[:n_layers],
        "ln2g": np.ascontiguousarray(_f32c(inputs["ln2_g"])[:, :, None])[:n_layers],
        "ln2b": np.ascontiguousarray(_f32c(inputs["ln2_b"])[:, :, None])[:n_layers],
        "lnfg": np.ascontiguousarray(_f32c(inputs["lnf_g"])[:, None]),
        "ln1g_row": np.ascontiguousarray(_f32c(inputs["ln1_g"])[:, None, :])[:n_layers],
        "ln2g_row": np.ascontiguousarray(_f32c(inputs["ln2_g"])[:, None, :])[:n_layers],
        "lnfg_row": np.ascontiguousarray(_f32c(inputs["lnf_g"])[None, :]),
        "lnfb": np.ascontiguousarray(_f32c(inputs["lnf_b"])[:, None]),
        "mask": _masks_for(r, not_pad),
    }
    spec16, spec32, spec8 = _pack_spec(n_layers)
    for nm, shp in spec16 + spec32 + spec8:
        assert tuple(vals[nm].shape) == tuple(shp), (nm, vals[nm].shape, shp)
    f8np = mybir.dt.np(f8)
    pk16 = np.concatenate(
        [np.ascontiguousarray(vals[nm]).astype(np.float16).ravel()
         for nm, _ in spec16])
    pk32 = np.concatenate(
        [np.ascontiguousarray(vals[nm]).astype(np.float32).ravel()
         for nm, _ in spec32])
    segs8 = [np.clip(np.ascontiguousarray(vals[nm]) * 256.0, -240.0, 240.0)
             .astype(f8np).ravel() for nm, _ in spec8 if vals[nm].size]
    pk8 = np.concatenate(segs8) if segs8 else np.zeros(1, f8np)
    return {"pk16": pk16, "pk32": pk32, "pk8": pk8}


def kernel(**inputs):
    global LAST_RESULTS
    n_layers = int(os.environ.get("KERNEL_LAYERS", L))
    taps = tuple(t for t in os.environ.get("KERNEL_TAPS", "").split(",") if t)
    key = (n_layers, taps)
    if key not in _CACHE:
        _CACHE[key] = build_nc(n_layers, taps)
    nc = _CACHE[key]
    in_maps = [_prep_core(inputs, c, n_layers) for c in range(NC)]
    res = bass_utils.run_bass_kernel_spmd(nc, in_maps, core_ids=list(range(NC)))
    LAST_RESULTS = res
    out = np.empty((B, S, V), np.float32)
    for g in range(B):
        for r in range(TP):
            out[g][:, r * VS:(r + 1) * VS] = res.results[g * TP + r][
                "logits"].astype(np.float32)
    return out

